# revision 52
# baseline (speedup 1.0000x reference)
"""Trainium2 Bass kernel for nn_DA_conv1D (dynamic depthwise conv1d + 1x1 conv
+ channel-attention gate), data-parallel over batch on 8 NeuronCores.

Shapes (hardcoded): x0 [32, 64, 16384] f32, x1 [32, 64] f32.
Each core handles 4 samples, organized as 2 "pairs" of 2 samples so the
128 SBUF partitions hold (2 samples x 64 channels).

Main pipeline, per pair, per 1024-wide group ([128 part, L free] layout):
  ps1 = sum_j diag(kern_j) @ x_shift_j   (PE, 2x3 accumulating bf16 matmuls,
                                          N=512 each: PSUM-bank limit;
                                          palindrome tap order d0,d1,d2,
                                          d2,d1,d0 hides LDWEIGHTS switches)
  lr  = lrelu(ps1)                       (ACT Prelu+bias, PSUM->SBUF, bf16)
  ps2 = blockdiag(conv_w) @ lr           (PE, 2x K=128 bf16 matmuls)
  out = x0 * att + ps2                   (DVE scalar_tensor_tensor, bf16 out)

PE is the bottleneck (4 passes/512 cols), so every other full-size chunk
computes its first group's depthwise on the otherwise-idle DVE instead
(chained per-partition-scalar mult-adds), with conv + att-residual via
accumulating PE matmuls (diag(att) pairs with blockdiag(conv_w)) and an
ACT Identity drain; that group's PE/ACT tail is emitted AFTER the chunk's
second (main) group so the strict-FIFO engine queues never stall on the
cross-engine round-trip.

Start-up: ~30 dummy matmuls on a memset tile warm the PE HAM clock-gate
(1.2->2.4 GHz needs ~3.4 us sustained busy) while the DMA queues do their
first-use ring-init; the diag-weight DMA is the scalar engine's first
instruction; the first chunks' output stores are deferred past the ramp.

x0 is pre-cast to bf16 on host (x + conv_b/att, folding the conv bias into
the gate) and DMAed once per chunk (matmul path + residual read share it);
the output returns as bf16 and is cast to fp32 on host (rel err ~7e-3 vs
2e-2 budget). The tiny dynamic-weight math (kern, att) runs on host in
fp32 and ships as per-core diagonal/gate tensors (a few hundred KB).
"""

import os
import sys

for _p in ("/opt/trn_rl_repo", "/root/.axon_site/_ro/trn_rl_repo"):
    if os.path.isdir(_p) and _p not in sys.path:
        sys.path.append(_p)

import ml_dtypes
import numpy as np

import concourse.bacc as bacc
import concourse.tile as tile
from concourse import mybir
from concourse.bass_utils import run_bass_kernel_spmd

B, C, L, K = 32, 64, 16384, 3
N_CORES = 8
SAMPLES_PER_CORE = B // N_CORES          # 4
PAIRS = SAMPLES_PER_CORE // 2            # 2
P = 128                                  # SBUF partitions = 2 samples x 64 ch
CHUNK = 2048                             # max chunk (SBUF tile size)
# tapered schedule: small chunks at the start of the first pair (shrink
# pipeline fill) and the end of the last pair (shrink drain); mid-stream
# chunks stay big
CHUNK_SCHED = [
    [512, 512, 1024] + [2048] * 7,
    [2048] * 7 + [1024, 512, 512],
]
MTILE = 512                              # PSUM bank width (fp32)
NTILE = 512                              # matmul moving width (PSUM bank)
# PE warm-up: dummy matmuls issued while the input DMAs fill SBUF, so the
# HAM clock-gate un-throttles (1.2->2.4 GHz) before the first real matmul
# (needs >=3.4 us of sustained PE busy, ending when the first chunk lands)
WARM_MMS = [128] * 16 + [256] * 6
# every other full-size chunk runs its first 1024-col group's depthwise on
# the Vector engine instead of PE, to shift load off the PE bottleneck
F32 = mybir.dt.float32
BF16 = mybir.dt.bfloat16
BF16_NP = ml_dtypes.bfloat16

TRACE = False          # test harness flips this to profile
USE_LRELU = True       # HW Prelu activation (CoreSim lacks it; see simcheck)
LAST_RESULT = None     # BassKernelResults of the most recent run

_COMPILED = {}         # (use_lrelu,) -> compiled Bacc program


def _build_program(use_lrelu):
    nc = bacc.Bacc("TRN2", target_bir_lowering=False, debug=False,
                   num_devices=N_CORES)

    x0b = nc.dram_tensor("x0b", [PAIRS, P, L], BF16,
                         kind="ExternalInput").ap()
    # diag kernels pre-flattened per partition: [(pair, tap) -> 128 cols]
    diags = nc.dram_tensor("diags", [P, PAIRS * K * P], BF16,
                           kind="ExternalInput").ap()
    # scal[:, 0:PAIRS] = att per pair; scal[:, PAIRS:2*PAIRS] = prelu bias
    # (-sum_j kern_j * d, the depthwise compensation for the host-side
    #  x0 + d shift that folds conv_b into the residual term);
    # scal[:, 2*PAIRS + p*K + j] = kern tap j of pair p (DVE alt path)
    scal = nc.dram_tensor("scal", [P, 2 * PAIRS + PAIRS * K], F32,
                          kind="ExternalInput").ap()
    # diag(att) per pair as bf16 matrices (PE residual on alt groups)
    adiag = nc.dram_tensor("adiag", [P, PAIRS * P], BF16,
                           kind="ExternalInput").ap()
    # bf16(d) per pair: halo fill value so padded taps cancel exactly
    dcol = nc.dram_tensor("dcol", [PAIRS, P, 1], BF16,
                          kind="ExternalInput").ap()
    wblk = nc.dram_tensor("wblk", [P, P], BF16, kind="ExternalInput").ap()
    out = nc.dram_tensor("out", [PAIRS, P, L], BF16, kind="ExternalOutput").ap()

    mult = mybir.AluOpType.mult
    add = mybir.AluOpType.add
    Relu = mybir.ActivationFunctionType.Relu
    Prelu = mybir.ActivationFunctionType.Prelu
    Ident = mybir.ActivationFunctionType.Identity

    with tile.TileContext(nc) as tc:
        with (
            tc.tile_pool(name="consts", bufs=1) as consts,
            tc.tile_pool(name="xbf", bufs=8) as xbf_pool,
            tc.tile_pool(name="lr", bufs=4) as lr_pool,
            tc.tile_pool(name="r9", bufs=4) as r9_pool,
            tc.tile_pool(name="sdw", bufs=2) as sdw_pool,
            tc.tile_pool(name="outc", bufs=6) as out_pool,
            tc.tile_pool(name="ps1", bufs=2, space="PSUM") as ps1_pool,
            tc.tile_pool(name="ps2", bufs=2, space="PSUM") as ps2_pool,
        ):
            # the diag weights gate the first real matmul: issue their DMA
            # as the scalar engine's very first instruction
            diag_t = consts.tile([P, PAIRS * K * P], BF16)
            nc.scalar.dma_start(diag_t[:], diags[:])

            # first chunk load issued before the rest so the input stream
            # starts immediately
            sz0 = CHUNK_SCHED[0][0]
            first_xbf = xbf_pool.tile([P, CHUNK + 4], BF16, tag="xbf")
            nc.sync.dma_start(first_xbf[:, 1:2], dcol[0])
            nc.sync.dma_start(first_xbf[:, 2:sz0 + 3],
                              x0b[0, :, 0:sz0 + 1])

            wblk_t = consts.tile([P, P], BF16)
            nc.scalar.dma_start(wblk_t[:], wblk[:])
            scal_t = consts.tile([P, 2 * PAIRS + PAIRS * K], F32)
            nc.scalar.dma_start(scal_t[:], scal[:])
            adiag_t = consts.tile([P, PAIRS * P], BF16)
            nc.scalar.dma_start(adiag_t[:], adiag[:])
            att = [scal_t[:, p:p + 1] for p in range(PAIRS)]
            pb = [scal_t[:, PAIRS + p:PAIRS + p + 1] for p in range(PAIRS)]
            kc = [[scal_t[:, 2 * PAIRS + p * K + j:2 * PAIRS + p * K + j + 1]
                   for j in range(K)] for p in range(PAIRS)]

            # PE warm-up on a memset tile (no DMA dependency): dummy
            # matmuls discarded into a scratch PSUM tile while the first
            # x chunk and diag weights stream in
            warm_t = consts.tile([P, 256], BF16)
            nc.vector.memset(warm_t[:], 0.0)
            ps_w = ps2_pool.tile([P, MTILE], F32, name="ps2")
            for wn in WARM_MMS:
                nc.tensor.matmul(ps_w[:, 0:wn], warm_t[:, 0:P],
                                 warm_t[:, 0:wn], start=True, stop=True)

            cc = [0]                     # global chunk counter (alt select)
            deferred_out = []            # early out-DMAs held past the ramp
            for p in range(PAIRS):
                lo = 0
                sched = CHUNK_SCHED[p]
                for c, csz in enumerate(sched):
                    # xbf[:, i] = x0[lo + i - 2]; i=0 never read
                    # chunk loads stay on sync: it has no compute work, and
                    # a DMA trigger on a busy engine's sequencer (~0.6 us)
                    # blocks that engine's dispatch (strict FIFO)
                    ldeng = nc.sync
                    if p == 0 and c == 0:
                        xbf = first_xbf
                    else:
                        xbf = xbf_pool.tile([P, CHUNK + 4], BF16, tag="xbf")
                        if c == 0:
                            ldeng.dma_start(xbf[:, 1:2], dcol[p])
                            ldeng.dma_start(xbf[:, 2:csz + 3],
                                            x0b[p, :, 0:csz + 1])
                        elif c == len(sched) - 1:
                            ldeng.dma_start(xbf[:, csz + 2:csz + 3],
                                            dcol[p])
                            ldeng.dma_start(xbf[:, 1:csz + 2],
                                            x0b[p, :, lo - 1:lo + csz])
                        else:
                            ldeng.dma_start(xbf[:, 1:csz + 3],
                                            x0b[p, :, lo - 1:lo + csz + 1])

                    outc = out_pool.tile([P, CHUNK], BF16, tag="outc")
                    nt = csz // MTILE
                    last_chunk = (p == PAIRS - 1 and c == len(sched) - 1)

                    def emit_main(u, W, p=p, xbf=xbf, outc=outc, lo=lo,
                                  last_chunk=last_chunk):
                        nh = W // NTILE
                        ps1 = ps1_pool.tile([P, W], F32, name="ps1")
                        for h in range(nh):
                            # palindrome tap order across halves shares the
                            # edge LDWEIGHTS (d0,d1,d2,d2,d1,d0) so fewer
                            # weight switches are exposed
                            taps = range(K) if h % 2 == 0 else \
                                range(K - 1, -1, -1)
                            for i, j in enumerate(taps):
                                nc.tensor.matmul(
                                    ps1[:, h * NTILE:(h + 1) * NTILE],
                                    diag_t[:, (p * K + j) * P:
                                           (p * K + j + 1) * P],
                                    xbf[:, u + h * NTILE + 1 + j:
                                        u + h * NTILE + 1 + j + NTILE],
                                    start=(i == 0), stop=(i == K - 1),
                                )
                        lr = lr_pool.tile([P, 2 * MTILE], BF16, name="lr")
                        if use_lrelu:
                            nc.scalar.activation(lr[:, :W], ps1[:], Prelu,
                                                 bias=pb[p], alpha=0.1)
                        else:
                            tt = r9_pool.tile([P, W], F32, tag="tt")
                            nc.scalar.activation(tt[:], ps1[:], Ident,
                                                 bias=pb[p])
                            r9 = r9_pool.tile([P, W], F32, name="r9")
                            nc.scalar.activation(r9[:], tt[:], Relu,
                                                 scale=0.9)
                            nc.vector.scalar_tensor_tensor(
                                lr[:, :W], tt[:], 0.1, r9[:],
                                op0=mult, op1=add)
                        ps2 = ps2_pool.tile([P, W], F32, name="ps2")
                        for h in range(nh):
                            hs = slice(h * NTILE, (h + 1) * NTILE)
                            nc.tensor.matmul(ps2[:, hs], wblk_t[:],
                                             lr[:, hs],
                                             start=True, stop=True)
                        nc.vector.scalar_tensor_tensor(
                            outc[:, u:u + W],
                            xbf[:, u + 2:u + 2 + W],
                            att[p], ps2[:], op0=mult, op1=add)
                        if last_chunk:
                            # per-tile store so the final DMA isn't
                            # serialized behind the whole chunk
                            nc.gpsimd.dma_start(
                                out[p, :, lo + u:lo + u + W],
                                outc[:, u:u + W])

                    # every other full-size chunk computes the depthwise of
                    # its FIRST half-group on the Vector engine (PE does
                    # only conv + att-residual there). The alt group's PE
                    # matmuls and its ACT drain are emitted AFTER the
                    # second (main) group so the strict-FIFO engine queues
                    # never stall on the cross-engine round-trip.
                    alt_chunk = (use_lrelu and csz == CHUNK
                                 and cc[0] % 2 == 1)
                    cc[0] += 1
                    if alt_chunk:
                        W = 2 * MTILE
                        # DVE depthwise chain for cols [0, W)
                        s1 = sdw_pool.tile([P, W], BF16, tag="s1")
                        nc.vector.tensor_scalar(
                            s1[:], xbf[:, 1:1 + W], kc[p][0], None, mult)
                        s2 = sdw_pool.tile([P, W], BF16, tag="s2")
                        nc.vector.scalar_tensor_tensor(
                            s2[:], xbf[:, 2:2 + W], kc[p][1], s1[:],
                            op0=mult, op1=add)
                        s3 = sdw_pool.tile([P, W], BF16, tag="s3")
                        nc.vector.scalar_tensor_tensor(
                            s3[:], xbf[:, 3:3 + W], kc[p][2], s2[:],
                            op0=mult, op1=add)
                        lra = lr_pool.tile([P, 2 * MTILE], BF16, name="lr")
                        nc.scalar.activation(lra[:, :W], s3[:], Prelu,
                                             bias=pb[p], alpha=0.1)
                        # second half: ordinary PE pipeline
                        emit_main(W, W)
                        # now the deferred PE tail for the first half
                        ps2a = ps2_pool.tile([P, W], F32, name="ps2")
                        asl = adiag_t[:, p * P:(p + 1) * P]
                        for h in range(W // NTILE):
                            hs = slice(h * NTILE, (h + 1) * NTILE)
                            nc.tensor.matmul(ps2a[:, hs], wblk_t[:],
                                             lra[:, hs],
                                             start=True, stop=False)
                            nc.tensor.matmul(
                                ps2a[:, hs], asl,
                                xbf[:, 2 + h * NTILE:2 + (h + 1) * NTILE],
                                start=False, stop=True)
                        nc.scalar.activation(outc[:, 0:W], ps2a[:], Ident)
                    else:
                        t0 = 0
                        while t0 < nt:
                            wide = t0 + 1 < nt
                            W = 2 * MTILE if wide else MTILE
                            emit_main(t0 * MTILE, W)
                            t0 += 2 if wide else 1
                    if not last_chunk:
                        if cc[0] <= 2:
                            # defer the first chunks' stores past the
                            # DMA-queue ramp so input DMAs get the early
                            # bandwidth
                            deferred_out.append(
                                (out[p, :, lo:lo + csz], outc[:, :csz]))
                        else:
                            nc.gpsimd.dma_start(out[p, :, lo:lo + csz],
                                                outc[:, :csz])
                            while deferred_out:
                                dst, dsrc = deferred_out.pop()
                                nc.gpsimd.dma_start(dst, dsrc)
                    lo += csz

    nc.compile()
    return nc


def _lrelu(x):
    return np.where(x >= 0, x, np.float32(0.1) * x)


def kernel(x0, x1, W1, W2, conv_w, conv_b, ca_w1, ca_w2):
    global LAST_RESULT
    x0 = np.ascontiguousarray(np.asarray(x0, dtype=np.float32))
    x1 = np.asarray(x1, dtype=np.float32)
    W1 = np.asarray(W1, dtype=np.float32)
    W2 = np.asarray(W2, dtype=np.float32)
    conv_w = np.asarray(conv_w, dtype=np.float32)
    conv_b = np.asarray(conv_b, dtype=np.float32)
    ca_w1 = np.asarray(ca_w1, dtype=np.float32)
    ca_w2 = np.asarray(ca_w2, dtype=np.float32)

    # dynamic depthwise kernels + SE gate (tiny, fp32 host math)
    h = _lrelu(x1 @ W1.T)                                   # [B, 64]
    kern = (h @ W2.T).reshape(B, C, K)                      # [B, C, K]
    att = 1.0 / (1.0 + np.exp(-(_lrelu(x1 @ ca_w1.T) @ ca_w2.T)))
    att = att.astype(np.float32)                            # [B, C]

    # block-diagonal 1x1-conv weight as lhsT: lhsT[k, m] = W[m, k]
    wblk_np = np.zeros((P, P), np.float32)
    wblk_np[:C, :C] = conv_w.T
    wblk_np[C:, C:] = conv_w.T
    wblk_np = wblk_np.astype(BF16_NP)

    key = (USE_LRELU,)
    if key not in _COMPILED:
        _COMPILED[key] = _build_program(USE_LRELU)
    nc = _COMPILED[key]

    biasP = np.tile(conv_b, 2).astype(np.float32)            # [P]
    in_maps = []
    for core in range(N_CORES):
        s0 = core * SAMPLES_PER_CORE
        diags_np = np.zeros((P, PAIRS * K * P), np.float32)
        adiag_np = np.zeros((P, PAIRS * P), np.float32)
        scal_np = np.empty((P, 2 * PAIRS + PAIRS * K), np.float32)
        dcol_np = np.empty((PAIRS, P, 1), np.float32)
        dvals = np.empty((PAIRS, P), np.float32)
        for p in range(PAIRS):
            ka = kern[s0 + 2 * p]          # [C, K]
            kb = kern[s0 + 2 * p + 1]
            kern_bf = np.empty((P, K), np.float32)
            for j in range(K):
                s = (p * K + j) * P
                d = np.concatenate([ka[:, j], kb[:, j]])
                np.fill_diagonal(diags_np[:, s:s + P], d)
                kern_bf[:, j] = d.astype(BF16_NP).astype(np.float32)
                scal_np[:, 2 * PAIRS + p * K + j] = kern_bf[:, j]
            attp = np.concatenate([att[s0 + 2 * p], att[s0 + 2 * p + 1]])
            np.fill_diagonal(adiag_np[:, p * P:(p + 1) * P], attp)
            dp = biasP / attp                                 # [P]
            dvals[p] = dp
            dcol_np[p, :, 0] = dp
            scal_np[:, p] = attp
            # depthwise compensation: -sum_j bf16(kern_j) * d
            scal_np[:, PAIRS + p] = -(kern_bf.sum(axis=1) * dp)
        x0c = x0[s0:s0 + SAMPLES_PER_CORE].reshape(PAIRS, P, L)
        x0c = (x0c + dvals[:, :, None]).astype(BF16_NP)
        in_maps.append({
            "x0b": x0c,
            "diags": diags_np.astype(BF16_NP),
            "adiag": adiag_np.astype(BF16_NP),
            "scal": scal_np,
            "dcol": dcol_np.astype(BF16_NP),
            "wblk": wblk_np,
        })

    res = run_bass_kernel_spmd(nc, in_maps, list(range(N_CORES)), trace=TRACE)
    LAST_RESULT = res

    full = np.empty((B, C, L), np.float32)
    for core in range(N_CORES):
        s0 = core * SAMPLES_PER_CORE
        full[s0:s0 + SAMPLES_PER_CORE] = (
            np.asarray(res.results[core]["out"])
            .astype(np.float32).reshape(SAMPLES_PER_CORE, C, L))
    return full

